# revision 14
# baseline (speedup 1.0000x reference)
"""Trainium2 Bass kernel for nn_CMDPEncoder (VQ codebook quantize + random
batch-mix dequantize + DP noise).

Reference semantics:
    dots = einsum('bsd,vd->bsv', base, codebook)
    qi   = argmin_v(csq[v] - 2*dots)                  # [B,S]
    codes[b,s,j] = qi[rand_idx[b,s,j], s]
    out  = mean_j codebook[codes] + 0.1*noise

Sharding: split the sequence dim S across the 8 cores (64 positions each).
The rand_idx mixing crosses only the batch dim at fixed s, so with S-sharding
every core's mixing is fully local (no collectives).  Tokens are laid out
s-major (t = s_local*16 + b) so each 128-token tile holds 8 complete
s-groups of 16 batches, and the mix becomes a block-diagonal [128,128]
matmul with host-precomputed weights (counts/4 from rand_idx).

Scoring runs on the tensor engine: scores = 2*dots - (csq-768), with the
csq term folded in as an extra K=2 contraction chunk in fp16 hi/lo pairs
(exact to ~6e-5; the min top-2 score gap on this data is ~2.2e-3).

Score matmul variants (VARIANT):
  fp32    - plain fp32 matmuls (4 cycles/row). Safe, slowest.
  fp16x3  - 3-term Dekker split 2x*c = xh*ch + xh*cl + xl*ch in fp16
            (1 cycle/row, 18 chunks). Error ~1e-5, safe, ~25% faster.
  fp32r   - single-pass float32r (1 cycle/row, 6 chunks) + exact top-2
            rescore/fixup on DVE. fp32r alone has ~2e-2 max dot error,
            so the top-2 candidates are rescored with exact fp32 dots and
            the winner picked from those. Fastest.

Argmax via DVE max/max_index, dequantize via gpsimd indirect DMA gather of
codebook rows, mix via a second matmul, noise added during the PSUM drain.
"""

import os
import sys

for p in ("/opt/trn_rl_repo",):
    if p not in sys.path:
        sys.path.insert(0, p)

import numpy as np

import concourse.bacc as bacc
import concourse.bass as bass
import concourse.mybir as mybir
import concourse.tile as tile
from concourse.bass_utils import run_bass_kernel_spmd

B, S, D, V, K = 16, 512, 768, 4096, 4
N_CORES = 8
SS = S // N_CORES            # 64 sequence positions per core
T = SS * B                   # 1024 tokens per core, t = s_local*16 + b
TT = T // 128                # 8 token tiles per core
KC = D // 128                # 6 contraction chunks
NV = V // 512                # 8 V-tiles
DP_EPSILON = 0.1
CSQ_CENTER = 768.0
DE = 776                     # padded cb_ext row: 768 cb + 1 csq + 7 pad

F32 = mybir.dt.float32
F32R = mybir.dt.float32r
F16 = mybir.dt.float16
BF16 = mybir.dt.bfloat16
U32 = mybir.dt.uint32
I32 = mybir.dt.int32

VARIANT = os.environ.get("CMDP_VARIANT", "bf16fix")

_CACHED = {}


def _is_fixup(variant):
    return variant.endswith("fix")


def _base(variant):
    return variant[:-3] if variant.endswith("fix") else variant


def _score_terms(variant):
    """[(lhs_tensor_name, rhs_tensor_name, dtype)] for the 6-chunk terms."""
    base = _base(variant)
    if base == "fp32":
        return [("xT", "cbT", F32)]
    if base == "fp16x3":
        return [("xTh", "cbTh", F16), ("xTh", "cbTl", F16), ("xTl", "cbTh", F16)]
    if base == "fp32r":
        return [("xT", "cbT", F32R)]
    if base == "bf16":
        return [("xTb", "cbTb", BF16)]
    raise ValueError(variant)


def _build_nc(variant):
    fixup = _is_fixup(variant)
    terms = _score_terms(variant)
    lhs_names = sorted({t[0] for t in terms})
    rhs_names = sorted({t[1] for t in terms})

    nc = bacc.Bacc("TRN2", target_bir_lowering=False, debug=False,
                   num_devices=N_CORES)

    lhs_d = {n: nc.dram_tensor(n, [D, T], dict(terms)[n] if False else
                               [t[2] for t in terms if t[0] == n][0],
                               kind="ExternalInput") for n in lhs_names}
    rhs_d = {n: nc.dram_tensor(n, [D, V],
                               [t[2] for t in terms if t[1] == n][0],
                               kind="ExternalInput") for n in rhs_names}
    cbe_d = nc.dram_tensor("cbe", [V, DE], F32, kind="ExternalInput")
    csqL_d = nc.dram_tensor("csqL", [2, T], F16, kind="ExternalInput")
    csqR_d = nc.dram_tensor("csqR", [2, V], F16, kind="ExternalInput")
    w_d = nc.dram_tensor("w", [TT, 128, 128], F32, kind="ExternalInput")
    noise_d = nc.dram_tensor("noise", [T, D], F32, kind="ExternalInput")
    if fixup:
        xn_d = nc.dram_tensor("xn", [T, D], F32, kind="ExternalInput")
    out_d = nc.dram_tensor("out", [T, D], F32, kind="ExternalOutput")

    with tile.TileContext(nc) as tc:
        with (
            tc.tile_pool(name="big", bufs=1) as big,
            tc.tile_pool(name="work", bufs=2) as work,
            tc.tile_pool(name="sc", bufs=3) as sc_pool,
            tc.tile_pool(name="ypool", bufs=3) as ypool,
            tc.tile_pool(name="io", bufs=3) as io,
            tc.tile_pool(name="ps_s", bufs=4, space="PSUM") as ps_s,
            tc.tile_pool(name="ps_m", bufs=2, space="PSUM") as ps_m,
        ):
            lhs_t = {}
            rhs_t = {}
            for n, d in lhs_d.items():
                tl = big.tile([128, KC * T], d.dtype, tag=n)
                for k in range(KC):
                    nc.sync.dma_start(tl[:, k * T:(k + 1) * T],
                                      d.ap()[k * 128:(k + 1) * 128, :])
                lhs_t[n] = tl
            for n, d in rhs_d.items():
                tl = big.tile([128, KC * V], d.dtype, tag=n)
                for k in range(KC):
                    nc.sync.dma_start(tl[:, k * V:(k + 1) * V],
                                      d.ap()[k * 128:(k + 1) * 128, :])
                rhs_t[n] = tl
            csql = big.tile([2, T], F16)
            csqr = big.tile([2, V], F16)
            nc.sync.dma_start(csql[:], csqL_d.ap())
            nc.sync.dma_start(csqr[:], csqR_d.ap())
            w = big.tile([128, TT * 128], F32)
            for t in range(TT):
                nc.sync.dma_start(w[:, t * 128:(t + 1) * 128], w_d.ap()[t])

            def emit_scoring(t):
                tsl = slice(t * 128, (t + 1) * 128)
                scores = sc_pool.tile([128, V], F32, tag="scores")
                for v in range(NV):
                    vsl = slice(v * 512, (v + 1) * 512)
                    ps = ps_s.tile([128, 512], F32, tag="ps_score")
                    i = 0
                    for (ln, rn, _dt) in terms:
                        for k in range(KC):
                            nc.tensor.matmul(
                                ps[:],
                                lhs_t[ln][:, k * T + t * 128:k * T + (t + 1) * 128],
                                rhs_t[rn][:, k * V + v * 512:k * V + (v + 1) * 512],
                                start=(i == 0), stop=False)
                            i += 1
                    nc.tensor.matmul(ps[:], csql[:, tsl], csqr[:, vsl],
                                     start=False, stop=True)
                    nc.scalar.copy(out=scores[:, vsl], in_=ps[:])
                return scores

            def emit_scan_fixup(t, scores):
                """argmax (+ exact top-2 rescore) -> gather y rows."""
                tsl = slice(t * 128, (t + 1) * 128)
                mx = work.tile([128, 8], F32, tag="mx")
                idx = work.tile([128, 8], U32, tag="idx")
                nc.vector.max(mx[:], scores[:])
                nc.vector.max_index(idx[:], mx[:], scores[:])

                if not fixup:
                    idx32 = work.tile([128, 1], I32, tag="idx32")
                    nc.vector.tensor_copy(idx32[:], idx[:, 0:1])
                else:
                    # exact top-2 rescore: s_j = csq[cand_j] - 2*x.cb[cand_j]
                    xn = io.tile([128, D], F32, tag="xn")
                    nc.sync.dma_start(xn[:], xn_d.ap()[tsl, :])
                    cand = []
                    for j in range(2):
                        cj = work.tile([128, 1], I32, tag=f"cand{j}")
                        nc.vector.tensor_copy(cj[:], idx[:, j:j + 1])
                        cand.append(cj)
                    sj = []
                    for j in range(2):
                        g = work.tile([128, DE], F32, tag=f"g{j}")
                        nc.gpsimd.indirect_dma_start(
                            out=g[:], out_offset=None, in_=cbe_d.ap(),
                            in_offset=bass.IndirectOffsetOnAxis(
                                ap=cand[j][:, :1], axis=0))
                        # NB: tensor_tensor_reduce hard-faults TRN2 here; use
                        # a gpsimd mul (DVE stays on the scans) + DVE reduce.
                        tmp = work.tile([128, D], F32, tag="rescore_tmp")
                        dj = work.tile([128, 1], F32, tag=f"d{j}")
                        nc.gpsimd.tensor_mul(tmp[:], xn[:], g[:, 0:D])
                        nc.vector.tensor_reduce(
                            dj[:], tmp[:], axis=mybir.AxisListType.X,
                            op=mybir.AluOpType.add)
                        s = work.tile([128, 1], F32, tag=f"s{j}")
                        # s = (dj * -2) + csq_cand
                        nc.vector.scalar_tensor_tensor(
                            out=s[:], in0=dj[:], scalar=-2.0, in1=g[:, D:D + 1],
                            op0=mybir.AluOpType.mult, op1=mybir.AluOpType.add)
                        sj.append(s)
                    flip = work.tile([128, 1], I32, tag="flip")
                    nc.vector.tensor_tensor(out=flip[:], in0=sj[1][:],
                                            in1=sj[0][:],
                                            op=mybir.AluOpType.is_lt)
                    idx32 = work.tile([128, 1], I32, tag="idx32")
                    nc.vector.tensor_copy(idx32[:], cand[0][:])
                    nc.vector.copy_predicated(idx32[:], flip[:], cand[1][:])

                y = ypool.tile([128, DE], F32, tag="y")
                nc.gpsimd.indirect_dma_start(
                    out=y[:], out_offset=None, in_=cbe_d.ap(),
                    in_offset=bass.IndirectOffsetOnAxis(ap=idx32[:, :1], axis=0))
                return y

            def emit_output(t, y):
                """mix matmul -> ACT drain -> noise accum-DMA -> store."""
                tsl = slice(t * 128, (t + 1) * 128)
                pm = ps_m.tile([128, D], F32, tag="pm")
                nc.tensor.matmul(pm[:, 0:512], w[:, tsl], y[:, 0:512],
                                 start=True, stop=True)
                nc.tensor.matmul(pm[:, 512:D], w[:, tsl], y[:, 512:D],
                                 start=True, stop=True)
                ob = io.tile([128, D], F32, tag="out")
                nc.scalar.copy(out=ob[:], in_=pm[:])
                # add DP noise inline in the DMA (SWDGE accumulate)
                nc.gpsimd.dma_start(out=ob[:], in_=noise_d.ap()[tsl, :],
                                    accum_op=mybir.AluOpType.add)
                nc.sync.dma_start(out_d.ap()[tsl, :], ob[:])

            # 2-deep software pipeline: PE's instruction stream is
            # score(0) score(1) score(2) mix(0) score(3) mix(1) ... so the
            # scan/fixup/gather chain of tile t overlaps scoring of t+1/t+2
            # and the PE never stalls on it.
            PIPE = 2
            pending = []
            for t in range(TT):
                scores = emit_scoring(t)
                y = emit_scan_fixup(t, scores)
                pending.append((t, y))
                if len(pending) > PIPE:
                    emit_output(*pending.pop(0))
            for item in pending:
                emit_output(*item)

    nc.compile()
    return nc


def _prep_inputs(variant, base_embeddings, codebook, rand_idx, noise):
    """Build the 8 per-core input maps (all host-side numpy)."""
    fixup = _is_fixup(variant)
    x = np.ascontiguousarray(base_embeddings, dtype=np.float32)
    cb = np.ascontiguousarray(codebook, dtype=np.float32)
    ridx = np.asarray(rand_idx)
    nz = np.asarray(noise, dtype=np.float32)

    csq = (cb * cb).sum(-1, dtype=np.float32)              # [V]
    cbe = np.zeros((V, DE), np.float32)
    cbe[:, :D] = cb
    cbe[:, D] = csq
    csqc = (csq - CSQ_CENTER).astype(np.float32)
    r1 = csqc.astype(np.float16)
    r2 = (csqc - r1.astype(np.float32)).astype(np.float16)
    csqR = np.ascontiguousarray(np.stack([r1, r2]))        # [2, V] fp16
    csqL = np.full((2, T), -1.0, np.float16)

    shared = {"cbe": cbe, "csqL": csqL, "csqR": csqR}
    cbT = np.ascontiguousarray(cb.T)                       # [D, V] fp32
    base = _base(variant)
    if base in ("fp32", "fp32r"):
        shared["cbT"] = cbT
    elif base == "bf16":
        import ml_dtypes
        shared["cbTb"] = cbT.astype(ml_dtypes.bfloat16)
    elif base == "fp16x3":
        cbh = cbT.astype(np.float16)
        cbl = (cbT - cbh.astype(np.float32)).astype(np.float16)
        shared["cbTh"] = cbh
        shared["cbTl"] = cbl

    in_maps = []
    for c in range(N_CORES):
        ssl = slice(c * SS, (c + 1) * SS)
        # tokens t = s_local*16 + b
        xc = x[:, ssl, :].transpose(1, 0, 2).reshape(T, D)
        xT2 = np.ascontiguousarray((2.0 * xc).T)           # [D, T] fp32
        nzc = np.ascontiguousarray(
            DP_EPSILON * nz[:, ssl, :].transpose(1, 0, 2).reshape(T, D))
        rc = ridx[:, ssl, :]                               # [B, SS, K]
        wm = np.zeros((TT, 128, 128), np.float32)
        for tt in range(TT):
            for g in range(8):
                s_local = tt * 8 + g
                r = rc[:, s_local, :]                      # [B, K] in [0,B)
                cnt = np.zeros((B, B), np.float32)         # [dst=b, src]
                for bdst in range(B):
                    np.add.at(cnt[bdst], r[bdst], 1.0)
                wm[tt, g * 16:(g + 1) * 16, g * 16:(g + 1) * 16] = cnt.T / K
        m = {"w": wm, "noise": nzc, **shared}
        if base in ("fp32", "fp32r"):
            m["xT"] = xT2
        elif base == "bf16":
            import ml_dtypes
            m["xTb"] = xT2.astype(ml_dtypes.bfloat16)
        elif base == "fp16x3":
            xh = xT2.astype(np.float16)
            xl = (xT2 - xh.astype(np.float32)).astype(np.float16)
            m["xTh"] = xh
            m["xTl"] = xl
        if fixup:
            m["xn"] = np.ascontiguousarray(xc)
        in_maps.append(m)
    return in_maps


def kernel(base_embeddings, codebook, rand_idx, noise, _results_out=None):
    variant = VARIANT
    if variant not in _CACHED:
        _CACHED[variant] = _build_nc(variant)
    nc = _CACHED[variant]
    in_maps = _prep_inputs(variant, base_embeddings, codebook, rand_idx, noise)
    res = run_bass_kernel_spmd(nc, in_maps, list(range(N_CORES)))
    if _results_out is not None:
        _results_out.append(res)
    outs = []
    for c in range(N_CORES):
        oc = res.results[c]["out"].reshape(SS, B, D).transpose(1, 0, 2)
        outs.append(oc)
    return np.ascontiguousarray(np.concatenate(outs, axis=1))


# revision 17
# speedup vs baseline: 1.0509x; 1.0509x over previous
"""Trainium2 Bass kernel for nn_CMDPEncoder (VQ codebook quantize + random
batch-mix dequantize + DP noise).

Reference semantics:
    dots = einsum('bsd,vd->bsv', base, codebook)
    qi   = argmin_v(csq[v] - 2*dots)                  # [B,S]
    codes[b,s,j] = qi[rand_idx[b,s,j], s]
    out  = mean_j codebook[codes] + 0.1*noise

Sharding: split the sequence dim S across the 8 cores (64 positions each).
The rand_idx mixing crosses only the batch dim at fixed s, so with S-sharding
every core's mixing is fully local (no collectives).  Tokens are laid out
s-major (t = s_local*16 + b) so each 128-token tile holds 8 complete
s-groups of 16 batches, and the mix becomes a block-diagonal [128,128]
matmul with host-precomputed weights (counts/4 from rand_idx).

Scoring runs on the tensor engine: scores = 2*dots - (csq-768), with the
csq term folded in as an extra K=2 contraction chunk in fp16 hi/lo pairs
(exact to ~6e-5; the min top-2 score gap on this data is ~2.2e-3).

Score matmul variants (VARIANT):
  fp32    - plain fp32 matmuls (4 cycles/row). Safe, slowest.
  fp16x3  - 3-term Dekker split 2x*c = xh*ch + xh*cl + xl*ch in fp16
            (1 cycle/row, 18 chunks). Error ~1e-5, safe, ~25% faster.
  fp32r   - single-pass float32r (1 cycle/row, 6 chunks) + exact top-2
            rescore/fixup on DVE. fp32r alone has ~2e-2 max dot error,
            so the top-2 candidates are rescored with exact fp32 dots and
            the winner picked from those. Fastest.

Argmax via DVE max/max_index, dequantize via gpsimd indirect DMA gather of
codebook rows, mix via a second matmul, noise added during the PSUM drain.
"""

import os
import sys

for p in ("/opt/trn_rl_repo",):
    if p not in sys.path:
        sys.path.insert(0, p)

import numpy as np

import concourse.bacc as bacc
import concourse.bass as bass
import concourse.mybir as mybir
import concourse.tile as tile
from concourse.bass_utils import run_bass_kernel_spmd

B, S, D, V, K = 16, 512, 768, 4096, 4
N_CORES = 8
SS = S // N_CORES            # 64 sequence positions per core
T = SS * B                   # 1024 tokens per core, t = s_local*16 + b
TT = T // 128                # 8 token tiles per core
KC = D // 128                # 6 contraction chunks
NV = V // 512                # 8 V-tiles
DP_EPSILON = 0.1
CSQ_CENTER = 768.0
DE = 776                     # padded cb_ext row: 768 cb + 1 csq + 7 pad

F32 = mybir.dt.float32
F32R = mybir.dt.float32r
F16 = mybir.dt.float16
BF16 = mybir.dt.bfloat16
U32 = mybir.dt.uint32
I32 = mybir.dt.int32

VARIANT = os.environ.get("CMDP_VARIANT", "bf16fix")

_CACHED = {}


def _is_fixup(variant):
    return variant.endswith("fix")


def _base(variant):
    return variant[:-3] if variant.endswith("fix") else variant


def _score_terms(variant):
    """[(lhs_tensor_name, rhs_tensor_name, dtype)] for the 6-chunk terms."""
    base = _base(variant)
    if base == "fp32":
        return [("xT", "cbT", F32)]
    if base == "fp16x3":
        return [("xTh", "cbTh", F16), ("xTh", "cbTl", F16), ("xTl", "cbTh", F16)]
    if base == "fp32r":
        return [("xT", "cbT", F32R)]
    if base == "bf16":
        return [("xTb", "cbTb", BF16)]
    raise ValueError(variant)


def _build_nc(variant):
    fixup = _is_fixup(variant)
    terms = _score_terms(variant)
    lhs_names = sorted({t[0] for t in terms})
    rhs_names = sorted({t[1] for t in terms})

    nc = bacc.Bacc("TRN2", target_bir_lowering=False, debug=False,
                   num_devices=N_CORES)

    lhs_d = {n: nc.dram_tensor(n, [D, T], dict(terms)[n] if False else
                               [t[2] for t in terms if t[0] == n][0],
                               kind="ExternalInput") for n in lhs_names}
    rhs_d = {n: nc.dram_tensor(n, [D, V],
                               [t[2] for t in terms if t[1] == n][0],
                               kind="ExternalInput") for n in rhs_names}
    cbe_d = nc.dram_tensor("cbe", [V, DE], F32, kind="ExternalInput")
    csqL_d = nc.dram_tensor("csqL", [2, T], F16, kind="ExternalInput")
    csqR_d = nc.dram_tensor("csqR", [2, V], F16, kind="ExternalInput")
    w_d = nc.dram_tensor("w", [TT, 128, 128], F32, kind="ExternalInput")
    noise_d = nc.dram_tensor("noise", [T, D], F32, kind="ExternalInput")
    if fixup:
        xn_d = nc.dram_tensor("xn", [T, D], F32, kind="ExternalInput")
    out_d = nc.dram_tensor("out", [T, D], F32, kind="ExternalOutput")

    with tile.TileContext(nc) as tc:
        with (
            tc.tile_pool(name="big", bufs=1) as big,
            tc.tile_pool(name="work", bufs=2) as work,
            tc.tile_pool(name="sc", bufs=3) as sc_pool,
            tc.tile_pool(name="ypool", bufs=4) as ypool,
            tc.tile_pool(name="io", bufs=3) as io,
            tc.tile_pool(name="ps_s", bufs=4, space="PSUM") as ps_s,
            tc.tile_pool(name="ps_m", bufs=2, space="PSUM") as ps_m,
        ):
            lhs_t = {}
            rhs_t = {}
            for n, d in lhs_d.items():
                tl = big.tile([128, KC * T], d.dtype, tag=n)
                for k in range(KC):
                    nc.sync.dma_start(tl[:, k * T:(k + 1) * T],
                                      d.ap()[k * 128:(k + 1) * 128, :])
                lhs_t[n] = tl
            for n, d in rhs_d.items():
                tl = big.tile([128, KC * V], d.dtype, tag=n)
                rhs_t[n] = tl
            csql = big.tile([2, T], F16)
            csqr = big.tile([2, V], F16)
            nc.sync.dma_start(csql[:], csqL_d.ap())
            nc.sync.dma_start(csqr[:], csqR_d.ap())
            # codebook chunks arrive in V-quarters so scoring can start
            # before the whole 6.3MB transposed codebook has landed
            QV = V // 4
            for q in range(4):
                for n, d in rhs_d.items():
                    tl = rhs_t[n]
                    for k in range(KC):
                        nc.sync.dma_start(
                            tl[:, k * V + q * QV:k * V + (q + 1) * QV],
                            d.ap()[k * 128:(k + 1) * 128, q * QV:(q + 1) * QV])
            w = big.tile([128, TT * 128], F32)
            for t in range(TT):
                nc.sync.dma_start(w[:, t * 128:(t + 1) * 128], w_d.ap()[t])

            def emit_scoring(t):
                tsl = slice(t * 128, (t + 1) * 128)
                scores = sc_pool.tile([128, V], F32, tag="scores")
                for v in range(NV):
                    vsl = slice(v * 512, (v + 1) * 512)
                    ps = ps_s.tile([128, 512], F32, tag="ps_score")
                    i = 0
                    for (ln, rn, _dt) in terms:
                        for k in range(KC):
                            nc.tensor.matmul(
                                ps[:],
                                lhs_t[ln][:, k * T + t * 128:k * T + (t + 1) * 128],
                                rhs_t[rn][:, k * V + v * 512:k * V + (v + 1) * 512],
                                start=(i == 0), stop=False)
                            i += 1
                    nc.tensor.matmul(ps[:], csql[:, tsl], csqr[:, vsl],
                                     start=False, stop=True)
                    nc.scalar.copy(out=scores[:, vsl], in_=ps[:])
                return scores

            def emit_scan_fixup(t, scores):
                """argmax (+ exact top-2 rescore) -> gather y rows."""
                tsl = slice(t * 128, (t + 1) * 128)
                mx = work.tile([128, 8], F32, tag="mx")
                idx = work.tile([128, 8], U32, tag="idx")
                nc.vector.max(mx[:], scores[:])
                nc.vector.max_index(idx[:], mx[:], scores[:])

                if not fixup:
                    idx32 = work.tile([128, 1], I32, tag="idx32")
                    nc.vector.tensor_copy(idx32[:], idx[:, 0:1])
                else:
                    # exact top-2 rescore: s_j = csq[cand_j] - 2*x.cb[cand_j]
                    xn = io.tile([128, D], F32, tag="xn")
                    # ACT's HWDGE ring, so this small load doesn't queue
                    # behind the bulk input DMAs on the sync ring
                    nc.scalar.dma_start(xn[:], xn_d.ap()[tsl, :])
                    cand = []
                    for j in range(2):
                        cj = work.tile([128, 1], I32, tag=f"cand{j}")
                        nc.vector.tensor_copy(cj[:], idx[:, j:j + 1])
                        cand.append(cj)
                    sj = []
                    for j in range(2):
                        g = work.tile([128, DE], F32, tag=f"g{j}")
                        nc.gpsimd.indirect_dma_start(
                            out=g[:], out_offset=None, in_=cbe_d.ap(),
                            in_offset=bass.IndirectOffsetOnAxis(
                                ap=cand[j][:, :1], axis=0))
                        # NB: tensor_tensor_reduce hard-faults TRN2 here; use
                        # a gpsimd mul (DVE stays on the scans) + DVE reduce.
                        tmp = work.tile([128, D], F32, tag="rescore_tmp")
                        dj = work.tile([128, 1], F32, tag=f"d{j}")
                        nc.gpsimd.tensor_mul(tmp[:], xn[:], g[:, 0:D])
                        nc.vector.tensor_reduce(
                            dj[:], tmp[:], axis=mybir.AxisListType.X,
                            op=mybir.AluOpType.add)
                        s = work.tile([128, 1], F32, tag=f"s{j}")
                        # s = (dj * -2) + csq_cand
                        nc.vector.scalar_tensor_tensor(
                            out=s[:], in0=dj[:], scalar=-2.0, in1=g[:, D:D + 1],
                            op0=mybir.AluOpType.mult, op1=mybir.AluOpType.add)
                        sj.append(s)
                    flip = work.tile([128, 1], I32, tag="flip")
                    nc.vector.tensor_tensor(out=flip[:], in0=sj[1][:],
                                            in1=sj[0][:],
                                            op=mybir.AluOpType.is_lt)
                    idx32 = work.tile([128, 1], I32, tag="idx32")
                    nc.vector.tensor_copy(idx32[:], cand[0][:])
                    nc.vector.copy_predicated(idx32[:], flip[:], cand[1][:])

                y = ypool.tile([128, DE], F32, tag="y")
                nc.gpsimd.indirect_dma_start(
                    out=y[:], out_offset=None, in_=cbe_d.ap(),
                    in_offset=bass.IndirectOffsetOnAxis(ap=idx32[:, :1], axis=0))
                return y

            def emit_output(t, y):
                """mix matmul -> ACT drain -> noise accum-DMA -> store."""
                tsl = slice(t * 128, (t + 1) * 128)
                pm = ps_m.tile([128, D], F32, tag="pm")
                nc.tensor.matmul(pm[:, 0:512], w[:, tsl], y[:, 0:512],
                                 start=True, stop=True)
                nc.tensor.matmul(pm[:, 512:D], w[:, tsl], y[:, 512:D],
                                 start=True, stop=True)
                ob = io.tile([128, D], F32, tag="out")
                nc.scalar.copy(out=ob[:], in_=pm[:])
                # add DP noise inline in the DMA (SWDGE accumulate)
                nc.gpsimd.dma_start(out=ob[:], in_=noise_d.ap()[tsl, :],
                                    accum_op=mybir.AluOpType.add)
                nc.sync.dma_start(out_d.ap()[tsl, :], ob[:])

            # 2-deep software pipeline: PE's instruction stream is
            # score(0) score(1) score(2) mix(0) score(3) mix(1) ... so the
            # scan/fixup/gather chain of tile t overlaps scoring of t+1/t+2
            # and the PE never stalls on it.
            PIPE = 3
            pending = []
            for t in range(TT):
                scores = emit_scoring(t)
                y = emit_scan_fixup(t, scores)
                pending.append((t, y))
                if len(pending) > PIPE:
                    emit_output(*pending.pop(0))
            for item in pending:
                emit_output(*item)

    nc.compile()
    return nc


def _prep_inputs(variant, base_embeddings, codebook, rand_idx, noise):
    """Build the 8 per-core input maps (all host-side numpy)."""
    fixup = _is_fixup(variant)
    x = np.ascontiguousarray(base_embeddings, dtype=np.float32)
    cb = np.ascontiguousarray(codebook, dtype=np.float32)
    ridx = np.asarray(rand_idx)
    nz = np.asarray(noise, dtype=np.float32)

    csq = (cb * cb).sum(-1, dtype=np.float32)              # [V]
    cbe = np.zeros((V, DE), np.float32)
    cbe[:, :D] = cb
    cbe[:, D] = csq
    csqc = (csq - CSQ_CENTER).astype(np.float32)
    r1 = csqc.astype(np.float16)
    r2 = (csqc - r1.astype(np.float32)).astype(np.float16)
    csqR = np.ascontiguousarray(np.stack([r1, r2]))        # [2, V] fp16
    csqL = np.full((2, T), -1.0, np.float16)

    shared = {"cbe": cbe, "csqL": csqL, "csqR": csqR}
    cbT = np.ascontiguousarray(cb.T)                       # [D, V] fp32
    base = _base(variant)
    if base in ("fp32", "fp32r"):
        shared["cbT"] = cbT
    elif base == "bf16":
        import ml_dtypes
        shared["cbTb"] = cbT.astype(ml_dtypes.bfloat16)
    elif base == "fp16x3":
        cbh = cbT.astype(np.float16)
        cbl = (cbT - cbh.astype(np.float32)).astype(np.float16)
        shared["cbTh"] = cbh
        shared["cbTl"] = cbl

    in_maps = []
    for c in range(N_CORES):
        ssl = slice(c * SS, (c + 1) * SS)
        # tokens t = s_local*16 + b
        xc = x[:, ssl, :].transpose(1, 0, 2).reshape(T, D)
        xT2 = np.ascontiguousarray((2.0 * xc).T)           # [D, T] fp32
        nzc = np.ascontiguousarray(
            DP_EPSILON * nz[:, ssl, :].transpose(1, 0, 2).reshape(T, D))
        rc = ridx[:, ssl, :]                               # [B, SS, K]
        wm = np.zeros((TT, 128, 128), np.float32)
        for tt in range(TT):
            for g in range(8):
                s_local = tt * 8 + g
                r = rc[:, s_local, :]                      # [B, K] in [0,B)
                cnt = np.zeros((B, B), np.float32)         # [dst=b, src]
                for bdst in range(B):
                    np.add.at(cnt[bdst], r[bdst], 1.0)
                wm[tt, g * 16:(g + 1) * 16, g * 16:(g + 1) * 16] = cnt.T / K
        m = {"w": wm, "noise": nzc, **shared}
        if base in ("fp32", "fp32r"):
            m["xT"] = xT2
        elif base == "bf16":
            import ml_dtypes
            m["xTb"] = xT2.astype(ml_dtypes.bfloat16)
        elif base == "fp16x3":
            xh = xT2.astype(np.float16)
            xl = (xT2 - xh.astype(np.float32)).astype(np.float16)
            m["xTh"] = xh
            m["xTl"] = xl
        if fixup:
            m["xn"] = np.ascontiguousarray(xc)
        in_maps.append(m)
    return in_maps


def kernel(base_embeddings, codebook, rand_idx, noise, _results_out=None):
    variant = VARIANT
    if variant not in _CACHED:
        _CACHED[variant] = _build_nc(variant)
    nc = _CACHED[variant]
    in_maps = _prep_inputs(variant, base_embeddings, codebook, rand_idx, noise)
    res = run_bass_kernel_spmd(nc, in_maps, list(range(N_CORES)))
    if _results_out is not None:
        _results_out.append(res)
    outs = []
    for c in range(N_CORES):
        oc = res.results[c]["out"].reshape(SS, B, D).transpose(1, 0, 2)
        outs.append(oc)
    return np.ascontiguousarray(np.concatenate(outs, axis=1))


# revision 18
# speedup vs baseline: 1.0731x; 1.0211x over previous
"""Trainium2 Bass kernel for nn_CMDPEncoder (VQ codebook quantize + random
batch-mix dequantize + DP noise).

Reference semantics:
    dots = einsum('bsd,vd->bsv', base, codebook)
    qi   = argmin_v(csq[v] - 2*dots)                  # [B,S]
    codes[b,s,j] = qi[rand_idx[b,s,j], s]
    out  = mean_j codebook[codes] + 0.1*noise

Sharding: split the sequence dim S across the 8 cores (64 positions each).
The rand_idx mixing crosses only the batch dim at fixed s, so with S-sharding
every core's mixing is fully local (no collectives).  Tokens are laid out
s-major (t = s_local*16 + b) so each 128-token tile holds 8 complete
s-groups of 16 batches, and the mix becomes a block-diagonal [128,128]
matmul with host-precomputed weights (counts/4 from rand_idx).

Scoring runs on the tensor engine: scores = 2*dots - (csq-768), with the
csq term folded in as an extra K=2 contraction chunk in fp16 hi/lo pairs
(exact to ~6e-5; the min top-2 score gap on this data is ~2.2e-3).

Score matmul variants (VARIANT):
  fp32    - plain fp32 matmuls (4 cycles/row). Safe, slowest.
  fp16x3  - 3-term Dekker split 2x*c = xh*ch + xh*cl + xl*ch in fp16
            (1 cycle/row, 18 chunks). Error ~1e-5, safe, ~25% faster.
  fp32r   - single-pass float32r (1 cycle/row, 6 chunks) + exact top-2
            rescore/fixup on DVE. fp32r alone has ~2e-2 max dot error,
            so the top-2 candidates are rescored with exact fp32 dots and
            the winner picked from those. Fastest.

Argmax via DVE max/max_index, dequantize via gpsimd indirect DMA gather of
codebook rows, mix via a second matmul, noise added during the PSUM drain.
"""

import os
import sys

for p in ("/opt/trn_rl_repo",):
    if p not in sys.path:
        sys.path.insert(0, p)

import numpy as np

import concourse.bacc as bacc
import concourse.bass as bass
import concourse.mybir as mybir
import concourse.tile as tile
from concourse.bass_utils import run_bass_kernel_spmd

B, S, D, V, K = 16, 512, 768, 4096, 4
N_CORES = 8
SS = S // N_CORES            # 64 sequence positions per core
T = SS * B                   # 1024 tokens per core, t = s_local*16 + b
TT = T // 128                # 8 token tiles per core
KC = D // 128                # 6 contraction chunks
NV = V // 512                # 8 V-tiles
DP_EPSILON = 0.1
CSQ_CENTER = 768.0
DE = 776                     # padded cb_ext row: 768 cb + 1 csq + 7 pad

F32 = mybir.dt.float32
F32R = mybir.dt.float32r
F16 = mybir.dt.float16
BF16 = mybir.dt.bfloat16
U32 = mybir.dt.uint32
I32 = mybir.dt.int32

VARIANT = os.environ.get("CMDP_VARIANT", "bf16fix")

_CACHED = {}


def _is_fixup(variant):
    return variant.endswith("fix")


def _base(variant):
    return variant[:-3] if variant.endswith("fix") else variant


def _score_terms(variant):
    """[(lhs_tensor_name, rhs_tensor_name, dtype)] for the 6-chunk terms."""
    base = _base(variant)
    if base == "fp32":
        return [("xT", "cbT", F32)]
    if base == "fp16x3":
        return [("xTh", "cbTh", F16), ("xTh", "cbTl", F16), ("xTl", "cbTh", F16)]
    if base == "fp32r":
        return [("xT", "cbT", F32R)]
    if base == "bf16":
        return [("xTb", "cbTb", BF16)]
    raise ValueError(variant)


def _build_nc(variant):
    fixup = _is_fixup(variant)
    terms = _score_terms(variant)
    lhs_names = sorted({t[0] for t in terms})
    rhs_names = sorted({t[1] for t in terms})

    nc = bacc.Bacc("TRN2", target_bir_lowering=False, debug=False,
                   num_devices=N_CORES)

    lhs_d = {n: nc.dram_tensor(n, [128, KC * T],
                               [t[2] for t in terms if t[0] == n][0],
                               kind="ExternalInput") for n in lhs_names}
    rhs_d = {n: nc.dram_tensor(n, [128, KC * V],
                               [t[2] for t in terms if t[1] == n][0],
                               kind="ExternalInput") for n in rhs_names}
    cbe_d = nc.dram_tensor("cbe", [V, DE], F32, kind="ExternalInput")
    csqL_d = nc.dram_tensor("csqL", [2, T], F16, kind="ExternalInput")
    csqR_d = nc.dram_tensor("csqR", [2, V], F16, kind="ExternalInput")
    w_d = nc.dram_tensor("w", [128, TT * 128], F32, kind="ExternalInput")
    noise_d = nc.dram_tensor("noise", [T, D], F32, kind="ExternalInput")
    if fixup:
        xn_d = nc.dram_tensor("xn", [128, TT * D], F32, kind="ExternalInput")
    out_d = nc.dram_tensor("out", [T, D], F32, kind="ExternalOutput")

    with tile.TileContext(nc) as tc:
        with (
            tc.tile_pool(name="big", bufs=1) as big,
            tc.tile_pool(name="work", bufs=2) as work,
            tc.tile_pool(name="sc", bufs=3) as sc_pool,
            tc.tile_pool(name="ypool", bufs=4) as ypool,
            tc.tile_pool(name="io", bufs=3) as io,
            tc.tile_pool(name="ps_s", bufs=4, space="PSUM") as ps_s,
            tc.tile_pool(name="ps_m", bufs=2, space="PSUM") as ps_m,
        ):
            # all bulk inputs are host-pre-tiled to [128, ...] so each is a
            # single large DMA (HWDGE issue is ~0.6us per dma_start)
            QW = KC * 1024  # columns per codebook quarter
            lhs_t = {}
            rhs_t = {}
            for n, d in lhs_d.items():
                tl = big.tile([128, KC * T], d.dtype, tag=n)
                nc.sync.dma_start(tl[:], d.ap())
                lhs_t[n] = tl
            csql = big.tile([2, T], F16)
            csqr = big.tile([2, V], F16)
            nc.sync.dma_start(csql[:], csqL_d.ap())
            nc.sync.dma_start(csqr[:], csqR_d.ap())
            for n, d in rhs_d.items():
                tl = big.tile([128, KC * V], d.dtype, tag=n)
                rhs_t[n] = tl
            # first codebook quarter before xn, rest after, so scoring can
            # start early and the fixup input is there by tile 0's scan
            for n, d in rhs_d.items():
                nc.sync.dma_start(rhs_t[n][:, 0:QW], d.ap()[:, 0:QW])
            if fixup:
                xn_all = big.tile([128, TT * D], F32)
                nc.sync.dma_start(xn_all[:], xn_d.ap())
            for q in range(1, 4):
                for n, d in rhs_d.items():
                    nc.sync.dma_start(rhs_t[n][:, q * QW:(q + 1) * QW],
                                      d.ap()[:, q * QW:(q + 1) * QW])
            w = big.tile([128, TT * 128], F32)
            nc.sync.dma_start(w[:], w_d.ap())

            def emit_scoring(t):
                tsl = slice(t * 128, (t + 1) * 128)
                scores = sc_pool.tile([128, V], F32, tag="scores")
                for v in range(NV):
                    vsl = slice(v * 512, (v + 1) * 512)
                    ps = ps_s.tile([128, 512], F32, tag="ps_score")
                    i = 0
                    rc0 = (v // 2) * QW + (v % 2) * 512
                    for (ln, rn, _dt) in terms:
                        for k in range(KC):
                            nc.tensor.matmul(
                                ps[:],
                                lhs_t[ln][:, k * T + t * 128:k * T + (t + 1) * 128],
                                rhs_t[rn][:, rc0 + k * 1024:rc0 + k * 1024 + 512],
                                start=(i == 0), stop=False)
                            i += 1
                    nc.tensor.matmul(ps[:], csql[:, tsl], csqr[:, vsl],
                                     start=False, stop=True)
                    nc.scalar.copy(out=scores[:, vsl], in_=ps[:])
                return scores

            def emit_scan_fixup(t, scores):
                """argmax (+ exact top-2 rescore) -> gather y rows."""
                tsl = slice(t * 128, (t + 1) * 128)
                mx = work.tile([128, 8], F32, tag="mx")
                idx = work.tile([128, 8], U32, tag="idx")
                nc.vector.max(mx[:], scores[:])
                nc.vector.max_index(idx[:], mx[:], scores[:])

                if not fixup:
                    idx32 = work.tile([128, 1], I32, tag="idx32")
                    nc.vector.tensor_copy(idx32[:], idx[:, 0:1])
                else:
                    # exact top-2 rescore: s_j = csq[cand_j] - 2*x.cb[cand_j]
                    xn = xn_all[:, t * D:(t + 1) * D]
                    cand = []
                    for j in range(2):
                        cj = work.tile([128, 1], I32, tag=f"cand{j}")
                        nc.vector.tensor_copy(cj[:], idx[:, j:j + 1])
                        cand.append(cj)
                    sj = []
                    for j in range(2):
                        g = work.tile([128, DE], F32, tag=f"g{j}")
                        nc.gpsimd.indirect_dma_start(
                            out=g[:], out_offset=None, in_=cbe_d.ap(),
                            in_offset=bass.IndirectOffsetOnAxis(
                                ap=cand[j][:, :1], axis=0))
                        # NB: tensor_tensor_reduce hard-faults TRN2 here;
                        # scalar_tensor_tensor with accum_out does not.
                        tmp = work.tile([128, D], F32, tag="rescore_tmp")
                        dj = work.tile([128, 1], F32, tag=f"d{j}")
                        nc.vector.scalar_tensor_tensor(
                            out=tmp[:], in0=xn, scalar=1.0, in1=g[:, 0:D],
                            op0=mybir.AluOpType.bypass,
                            op1=mybir.AluOpType.mult, accum_out=dj[:])
                        s = work.tile([128, 1], F32, tag=f"s{j}")
                        # s = (dj * -2) + csq_cand
                        nc.vector.scalar_tensor_tensor(
                            out=s[:], in0=dj[:], scalar=-2.0, in1=g[:, D:D + 1],
                            op0=mybir.AluOpType.mult, op1=mybir.AluOpType.add)
                        sj.append(s)
                    flip = work.tile([128, 1], I32, tag="flip")
                    nc.vector.tensor_tensor(out=flip[:], in0=sj[1][:],
                                            in1=sj[0][:],
                                            op=mybir.AluOpType.is_lt)
                    idx32 = work.tile([128, 1], I32, tag="idx32")
                    nc.vector.tensor_copy(idx32[:], cand[0][:])
                    nc.vector.copy_predicated(idx32[:], flip[:], cand[1][:])

                y = ypool.tile([128, DE], F32, tag="y")
                nc.gpsimd.indirect_dma_start(
                    out=y[:], out_offset=None, in_=cbe_d.ap(),
                    in_offset=bass.IndirectOffsetOnAxis(ap=idx32[:, :1], axis=0))
                return y

            def emit_output(t, y):
                """mix matmul -> ACT drain -> noise accum-DMA -> store."""
                tsl = slice(t * 128, (t + 1) * 128)
                pm = ps_m.tile([128, D], F32, tag="pm")
                nc.tensor.matmul(pm[:, 0:512], w[:, tsl], y[:, 0:512],
                                 start=True, stop=True)
                nc.tensor.matmul(pm[:, 512:D], w[:, tsl], y[:, 512:D],
                                 start=True, stop=True)
                ob = io.tile([128, D], F32, tag="out")
                nc.scalar.copy(out=ob[:], in_=pm[:])
                # add DP noise inline in the DMA (SWDGE accumulate)
                nc.gpsimd.dma_start(out=ob[:], in_=noise_d.ap()[tsl, :],
                                    accum_op=mybir.AluOpType.add)
                nc.sync.dma_start(out_d.ap()[tsl, :], ob[:])

            # 2-deep software pipeline: PE's instruction stream is
            # score(0) score(1) score(2) mix(0) score(3) mix(1) ... so the
            # scan/fixup/gather chain of tile t overlaps scoring of t+1/t+2
            # and the PE never stalls on it.
            PIPE = 3
            pending = []
            for t in range(TT):
                scores = emit_scoring(t)
                y = emit_scan_fixup(t, scores)
                pending.append((t, y))
                if len(pending) > PIPE:
                    emit_output(*pending.pop(0))
            for item in pending:
                emit_output(*item)

    nc.compile()
    return nc


def _prep_inputs(variant, base_embeddings, codebook, rand_idx, noise):
    """Build the 8 per-core input maps (all host-side numpy)."""
    fixup = _is_fixup(variant)
    x = np.ascontiguousarray(base_embeddings, dtype=np.float32)
    cb = np.ascontiguousarray(codebook, dtype=np.float32)
    ridx = np.asarray(rand_idx)
    nz = np.asarray(noise, dtype=np.float32)

    csq = (cb * cb).sum(-1, dtype=np.float32)              # [V]
    cbe = np.zeros((V, DE), np.float32)
    cbe[:, :D] = cb
    cbe[:, D] = csq
    csqc = (csq - CSQ_CENTER).astype(np.float32)
    r1 = csqc.astype(np.float16)
    r2 = (csqc - r1.astype(np.float32)).astype(np.float16)
    csqR = np.ascontiguousarray(np.stack([r1, r2]))        # [2, V] fp16
    csqL = np.full((2, T), -1.0, np.float16)

    shared = {"cbe": cbe, "csqL": csqL, "csqR": csqR}
    # pre-tile [D, V] -> [128, (q, k, 1024)] quarters-major layout
    cbT = cb.T.reshape(KC, 128, 4, 1024).transpose(1, 2, 0, 3).reshape(128, KC * V)
    cbT = np.ascontiguousarray(cbT)
    base = _base(variant)
    if base in ("fp32", "fp32r"):
        shared["cbT"] = cbT
    elif base == "bf16":
        import ml_dtypes
        shared["cbTb"] = cbT.astype(ml_dtypes.bfloat16)
    elif base == "fp16x3":
        cbh = cbT.astype(np.float16)
        cbl = (cbT - cbh.astype(np.float32)).astype(np.float16)
        shared["cbTh"] = cbh
        shared["cbTl"] = cbl

    in_maps = []
    for c in range(N_CORES):
        ssl = slice(c * SS, (c + 1) * SS)
        # tokens t = s_local*16 + b
        xc = x[:, ssl, :].transpose(1, 0, 2).reshape(T, D)
        xT2 = (2.0 * xc).T                                 # [D, T] fp32
        # pre-tile [D, T] -> [128, KC*T]
        xT2 = np.ascontiguousarray(
            xT2.reshape(KC, 128, T).transpose(1, 0, 2).reshape(128, KC * T))
        nzc = np.ascontiguousarray(
            DP_EPSILON * nz[:, ssl, :].transpose(1, 0, 2).reshape(T, D))
        rc = ridx[:, ssl, :]                               # [B, SS, K]
        wm = np.zeros((TT, 128, 128), np.float32)
        for tt in range(TT):
            for g in range(8):
                s_local = tt * 8 + g
                r = rc[:, s_local, :]                      # [B, K] in [0,B)
                cnt = np.zeros((B, B), np.float32)         # [dst=b, src]
                for bdst in range(B):
                    np.add.at(cnt[bdst], r[bdst], 1.0)
                wm[tt, g * 16:(g + 1) * 16, g * 16:(g + 1) * 16] = cnt.T / K
        wm_t = np.ascontiguousarray(
            wm.transpose(1, 0, 2).reshape(128, TT * 128))
        m = {"w": wm_t, "noise": nzc, **shared}
        if base in ("fp32", "fp32r"):
            m["xT"] = xT2
        elif base == "bf16":
            import ml_dtypes
            m["xTb"] = xT2.astype(ml_dtypes.bfloat16)
        elif base == "fp16x3":
            xh = xT2.astype(np.float16)
            xl = (xT2 - xh.astype(np.float32)).astype(np.float16)
            m["xTh"] = xh
            m["xTl"] = xl
        if fixup:
            m["xn"] = np.ascontiguousarray(
                xc.reshape(TT, 128, D).transpose(1, 0, 2).reshape(128, TT * D))
        in_maps.append(m)
    return in_maps


def kernel(base_embeddings, codebook, rand_idx, noise, _results_out=None):
    variant = VARIANT
    if variant not in _CACHED:
        _CACHED[variant] = _build_nc(variant)
    nc = _CACHED[variant]
    in_maps = _prep_inputs(variant, base_embeddings, codebook, rand_idx, noise)
    res = run_bass_kernel_spmd(nc, in_maps, list(range(N_CORES)))
    if _results_out is not None:
        _results_out.append(res)
    outs = []
    for c in range(N_CORES):
        oc = res.results[c]["out"].reshape(SS, B, D).transpose(1, 0, 2)
        outs.append(oc)
    return np.ascontiguousarray(np.concatenate(outs, axis=1))


# revision 20
# speedup vs baseline: 1.1525x; 1.0741x over previous
"""Trainium2 Bass kernel for nn_CMDPEncoder (VQ codebook quantize + random
batch-mix dequantize + DP noise).

Reference semantics:
    dots = einsum('bsd,vd->bsv', base, codebook)
    qi   = argmin_v(csq[v] - 2*dots)                  # [B,S]
    codes[b,s,j] = qi[rand_idx[b,s,j], s]
    out  = mean_j codebook[codes] + 0.1*noise

Sharding: split the sequence dim S across the 8 cores (64 positions each).
The rand_idx mixing crosses only the batch dim at fixed s, so with S-sharding
every core's mixing is fully local (no collectives).  Tokens are laid out
s-major (t = s_local*16 + b) so each 128-token tile holds 8 complete
s-groups of 16 batches, and the mix becomes a block-diagonal [128,128]
matmul with host-precomputed weights (counts/4 from rand_idx).

Scoring runs on the tensor engine: scores = 2*dots - (csq-768), with the
csq term folded in as an extra K=2 contraction chunk in fp16 hi/lo pairs
(exact to ~6e-5; the min top-2 score gap on this data is ~2.2e-3).

Score matmul variants (VARIANT):
  fp32    - plain fp32 matmuls (4 cycles/row). Safe, slowest.
  fp16x3  - 3-term Dekker split 2x*c = xh*ch + xh*cl + xl*ch in fp16
            (1 cycle/row, 18 chunks). Error ~1e-5, safe, ~25% faster.
  fp32r   - single-pass float32r (1 cycle/row, 6 chunks) + exact top-2
            rescore/fixup on DVE. fp32r alone has ~2e-2 max dot error,
            so the top-2 candidates are rescored with exact fp32 dots and
            the winner picked from those. Fastest.

Argmax via DVE max/max_index, dequantize via gpsimd indirect DMA gather of
codebook rows, mix via a second matmul, noise added during the PSUM drain.
"""

import os
import sys

for p in ("/opt/trn_rl_repo",):
    if p not in sys.path:
        sys.path.insert(0, p)

import numpy as np

import concourse.bacc as bacc
import concourse.bass as bass
import concourse.mybir as mybir
import concourse.tile as tile
from concourse.bass_utils import run_bass_kernel_spmd

B, S, D, V, K = 16, 512, 768, 4096, 4
N_CORES = 8
SS = S // N_CORES            # 64 sequence positions per core
T = SS * B                   # 1024 tokens per core, t = s_local*16 + b
TT = T // 128                # 8 token tiles per core
KC = D // 128                # 6 contraction chunks
NV = V // 512                # 8 V-tiles
DP_EPSILON = 0.1
CSQ_CENTER = 768.0
DE = 776                     # padded cb_ext row: 768 cb + 1 csq + 7 pad

F32 = mybir.dt.float32
F32R = mybir.dt.float32r
F16 = mybir.dt.float16
BF16 = mybir.dt.bfloat16
U32 = mybir.dt.uint32
I32 = mybir.dt.int32

VARIANT = os.environ.get("CMDP_VARIANT", "bf16fix")

_CACHED = {}


def _is_fixup(variant):
    return variant.endswith("fix")


def _base(variant):
    return variant[:-3] if variant.endswith("fix") else variant


def _score_terms(variant):
    """[(lhs_tensor_name, rhs_tensor_name, dtype)] for the 6-chunk terms."""
    base = _base(variant)
    if base == "fp32":
        return [("xT", "cbT", F32)]
    if base == "fp16x3":
        return [("xTh", "cbTh", F16), ("xTh", "cbTl", F16), ("xTl", "cbTh", F16)]
    if base == "fp32r":
        return [("xT", "cbT", F32R)]
    if base == "bf16":
        return [("xTb", "cbTb", BF16)]
    raise ValueError(variant)


def _build_nc(variant):
    fixup = _is_fixup(variant)
    terms = _score_terms(variant)
    lhs_names = sorted({t[0] for t in terms})
    rhs_names = sorted({t[1] for t in terms})

    nc = bacc.Bacc("TRN2", target_bir_lowering=False, debug=False,
                   num_devices=N_CORES)

    lhs_d = {n: nc.dram_tensor(n, [128, KC * T],
                               [t[2] for t in terms if t[0] == n][0],
                               kind="ExternalInput") for n in lhs_names}
    rhs_d = {n: nc.dram_tensor(n, [128, KC * V],
                               [t[2] for t in terms if t[1] == n][0],
                               kind="ExternalInput") for n in rhs_names}
    cbe_d = nc.dram_tensor("cbe", [V, DE], F32, kind="ExternalInput")
    csqL_d = nc.dram_tensor("csqL", [2, T], F16, kind="ExternalInput")
    csqR_d = nc.dram_tensor("csqR", [2, V], F16, kind="ExternalInput")
    w_d = nc.dram_tensor("w", [128, TT * 128], F32, kind="ExternalInput")
    noise_d = nc.dram_tensor("noise", [T, D], F32, kind="ExternalInput")
    if fixup:
        xn_d = nc.dram_tensor("xn", [128, TT * D], F32, kind="ExternalInput")
    out_d = nc.dram_tensor("out", [T, D], F32, kind="ExternalOutput")

    with tile.TileContext(nc) as tc:
        with (
            tc.tile_pool(name="big", bufs=1) as big,
            tc.tile_pool(name="work", bufs=2) as work,
            tc.tile_pool(name="sc", bufs=3) as sc_pool,
            tc.tile_pool(name="ypool", bufs=4) as ypool,
            tc.tile_pool(name="io", bufs=3) as io,
            tc.tile_pool(name="ps_s", bufs=4, space="PSUM") as ps_s,
            tc.tile_pool(name="ps_m", bufs=2, space="PSUM") as ps_m,
        ):
            # all bulk inputs are host-pre-tiled to [128, ...] so each is a
            # single large DMA (HWDGE issue is ~0.6us per dma_start)
            QW = KC * 1024  # columns per codebook quarter
            lhs_t = {}
            rhs_t = {}
            for n, d in lhs_d.items():
                tl = big.tile([128, KC * T], d.dtype, tag=n)
                nc.sync.dma_start(tl[:], d.ap())
                lhs_t[n] = tl
            csql = big.tile([2, T], F16)
            csqr = big.tile([2, V], F16)
            nc.sync.dma_start(csql[:], csqL_d.ap())
            nc.sync.dma_start(csqr[:], csqR_d.ap())
            for n, d in rhs_d.items():
                tl = big.tile([128, KC * V], d.dtype, tag=n)
                rhs_t[n] = tl
            # whole codebook streams in before xn: tile 0's scoring
            # consumes all four quarters within its ~14us
            for q in range(4):
                for n, d in rhs_d.items():
                    nc.sync.dma_start(rhs_t[n][:, q * QW:(q + 1) * QW],
                                      d.ap()[:, q * QW:(q + 1) * QW])
            if fixup:
                xn_all = big.tile([128, TT * D], F32)
                nc.sync.dma_start(xn_all[:], xn_d.ap())
            w = big.tile([128, TT * 128], F32)
            nc.sync.dma_start(w[:], w_d.ap())

            def emit_scoring(t):
                tsl = slice(t * 128, (t + 1) * 128)
                scores = sc_pool.tile([128, V], F32, tag="scores")
                for v in range(NV):
                    vsl = slice(v * 512, (v + 1) * 512)
                    ps = ps_s.tile([128, 512], F32, tag="ps_score")
                    i = 0
                    rc0 = (v // 2) * QW + (v % 2) * 512
                    for (ln, rn, _dt) in terms:
                        for k in range(KC):
                            nc.tensor.matmul(
                                ps[:],
                                lhs_t[ln][:, k * T + t * 128:k * T + (t + 1) * 128],
                                rhs_t[rn][:, rc0 + k * 1024:rc0 + k * 1024 + 512],
                                start=(i == 0), stop=False)
                            i += 1
                    nc.tensor.matmul(ps[:], csql[:, tsl], csqr[:, vsl],
                                     start=False, stop=True)
                    nc.scalar.copy(out=scores[:, vsl], in_=ps[:])
                return scores

            def emit_scan_fixup(t, scores):
                """argmax (+ exact top-2 rescore) -> gather y rows."""
                tsl = slice(t * 128, (t + 1) * 128)
                mx = work.tile([128, 8], F32, tag="mx")
                idx = work.tile([128, 8], U32, tag="idx")
                nc.vector.max(mx[:], scores[:])
                nc.vector.max_index(idx[:], mx[:], scores[:])

                if not fixup:
                    idx32 = work.tile([128, 1], I32, tag="idx32")
                    nc.vector.tensor_copy(idx32[:], idx[:, 0:1])
                else:
                    # exact top-2 rescore: s_j = csq[cand_j] - 2*x.cb[cand_j]
                    xn = xn_all[:, t * D:(t + 1) * D]
                    cand = []
                    for j in range(2):
                        cj = work.tile([128, 1], I32, tag=f"cand{j}")
                        nc.vector.tensor_copy(cj[:], idx[:, j:j + 1])
                        cand.append(cj)
                    sj = []
                    for j in range(2):
                        g = work.tile([128, DE], F32, tag=f"g{j}")
                        nc.gpsimd.indirect_dma_start(
                            out=g[:], out_offset=None, in_=cbe_d.ap(),
                            in_offset=bass.IndirectOffsetOnAxis(
                                ap=cand[j][:, :1], axis=0))
                        # NB: tensor_tensor_reduce hard-faults TRN2 here;
                        # scalar_tensor_tensor with accum_out does not.
                        tmp = work.tile([128, D], F32, tag="rescore_tmp")
                        dj = work.tile([128, 1], F32, tag=f"d{j}")
                        nc.vector.scalar_tensor_tensor(
                            out=tmp[:], in0=xn, scalar=1.0, in1=g[:, 0:D],
                            op0=mybir.AluOpType.bypass,
                            op1=mybir.AluOpType.mult, accum_out=dj[:])
                        s = work.tile([128, 1], F32, tag=f"s{j}")
                        # s = (dj * -2) + csq_cand
                        nc.vector.scalar_tensor_tensor(
                            out=s[:], in0=dj[:], scalar=-2.0, in1=g[:, D:D + 1],
                            op0=mybir.AluOpType.mult, op1=mybir.AluOpType.add)
                        sj.append(s)
                    flip = work.tile([128, 1], I32, tag="flip")
                    nc.vector.tensor_tensor(out=flip[:], in0=sj[1][:],
                                            in1=sj[0][:],
                                            op=mybir.AluOpType.is_lt)
                    idx32 = work.tile([128, 1], I32, tag="idx32")
                    nc.vector.tensor_copy(idx32[:], cand[0][:])
                    nc.vector.copy_predicated(idx32[:], flip[:], cand[1][:])

                y = ypool.tile([128, DE], F32, tag="y")
                nc.gpsimd.indirect_dma_start(
                    out=y[:], out_offset=None, in_=cbe_d.ap(),
                    in_offset=bass.IndirectOffsetOnAxis(ap=idx32[:, :1], axis=0))
                return y

            def emit_output(t, y):
                """mix matmul -> ACT drain -> noise accum-DMA -> store."""
                tsl = slice(t * 128, (t + 1) * 128)
                pm = ps_m.tile([128, D], F32, tag="pm")
                nc.tensor.matmul(pm[:, 0:512], w[:, tsl], y[:, 0:512],
                                 start=True, stop=True)
                nc.tensor.matmul(pm[:, 512:D], w[:, tsl], y[:, 512:D],
                                 start=True, stop=True)
                ob = io.tile([128, D], F32, tag="out")
                nc.scalar.copy(out=ob[:], in_=pm[:])
                # add DP noise inline in the DMA (SWDGE accumulate)
                nc.gpsimd.dma_start(out=ob[:], in_=noise_d.ap()[tsl, :],
                                    accum_op=mybir.AluOpType.add)
                nc.sync.dma_start(out_d.ap()[tsl, :], ob[:])

            # 2-deep software pipeline: PE's instruction stream is
            # score(0) score(1) score(2) mix(0) score(3) mix(1) ... so the
            # scan/fixup/gather chain of tile t overlaps scoring of t+1/t+2
            # and the PE never stalls on it.
            PIPE = 3
            pending = []
            for t in range(TT):
                scores = emit_scoring(t)
                y = emit_scan_fixup(t, scores)
                pending.append((t, y))
                if len(pending) > PIPE:
                    emit_output(*pending.pop(0))
            for item in pending:
                emit_output(*item)

    nc.compile()
    return nc


def _prep_inputs(variant, base_embeddings, codebook, rand_idx, noise):
    """Build the 8 per-core input maps (all host-side numpy)."""
    fixup = _is_fixup(variant)
    x = np.ascontiguousarray(base_embeddings, dtype=np.float32)
    cb = np.ascontiguousarray(codebook, dtype=np.float32)
    ridx = np.asarray(rand_idx)
    nz = np.asarray(noise, dtype=np.float32)

    csq = (cb * cb).sum(-1, dtype=np.float32)              # [V]
    cbe = np.zeros((V, DE), np.float32)
    cbe[:, :D] = cb
    cbe[:, D] = csq
    csqc = (csq - CSQ_CENTER).astype(np.float32)
    r1 = csqc.astype(np.float16)
    r2 = (csqc - r1.astype(np.float32)).astype(np.float16)
    csqR = np.ascontiguousarray(np.stack([r1, r2]))        # [2, V] fp16
    csqL = np.full((2, T), -1.0, np.float16)

    shared = {"cbe": cbe, "csqL": csqL, "csqR": csqR}
    # pre-tile [D, V] -> [128, (q, k, 1024)] quarters-major layout
    cbT = cb.T.reshape(KC, 128, 4, 1024).transpose(1, 2, 0, 3).reshape(128, KC * V)
    cbT = np.ascontiguousarray(cbT)
    base = _base(variant)
    if base in ("fp32", "fp32r"):
        shared["cbT"] = cbT
    elif base == "bf16":
        import ml_dtypes
        shared["cbTb"] = cbT.astype(ml_dtypes.bfloat16)
    elif base == "fp16x3":
        cbh = cbT.astype(np.float16)
        cbl = (cbT - cbh.astype(np.float32)).astype(np.float16)
        shared["cbTh"] = cbh
        shared["cbTl"] = cbl

    in_maps = []
    for c in range(N_CORES):
        ssl = slice(c * SS, (c + 1) * SS)
        # tokens t = s_local*16 + b
        xc = x[:, ssl, :].transpose(1, 0, 2).reshape(T, D)
        xT2 = (2.0 * xc).T                                 # [D, T] fp32
        # pre-tile [D, T] -> [128, KC*T]
        xT2 = np.ascontiguousarray(
            xT2.reshape(KC, 128, T).transpose(1, 0, 2).reshape(128, KC * T))
        nzc = np.ascontiguousarray(
            DP_EPSILON * nz[:, ssl, :].transpose(1, 0, 2).reshape(T, D))
        rc = ridx[:, ssl, :]                               # [B, SS, K]
        wm = np.zeros((TT, 128, 128), np.float32)
        for tt in range(TT):
            for g in range(8):
                s_local = tt * 8 + g
                r = rc[:, s_local, :]                      # [B, K] in [0,B)
                cnt = np.zeros((B, B), np.float32)         # [dst=b, src]
                for bdst in range(B):
                    np.add.at(cnt[bdst], r[bdst], 1.0)
                wm[tt, g * 16:(g + 1) * 16, g * 16:(g + 1) * 16] = cnt.T / K
        wm_t = np.ascontiguousarray(
            wm.transpose(1, 0, 2).reshape(128, TT * 128))
        m = {"w": wm_t, "noise": nzc, **shared}
        if base in ("fp32", "fp32r"):
            m["xT"] = xT2
        elif base == "bf16":
            import ml_dtypes
            m["xTb"] = xT2.astype(ml_dtypes.bfloat16)
        elif base == "fp16x3":
            xh = xT2.astype(np.float16)
            xl = (xT2 - xh.astype(np.float32)).astype(np.float16)
            m["xTh"] = xh
            m["xTl"] = xl
        if fixup:
            m["xn"] = np.ascontiguousarray(
                xc.reshape(TT, 128, D).transpose(1, 0, 2).reshape(128, TT * D))
        in_maps.append(m)
    return in_maps


def kernel(base_embeddings, codebook, rand_idx, noise, _results_out=None):
    variant = VARIANT
    if variant not in _CACHED:
        _CACHED[variant] = _build_nc(variant)
    nc = _CACHED[variant]
    in_maps = _prep_inputs(variant, base_embeddings, codebook, rand_idx, noise)
    res = run_bass_kernel_spmd(nc, in_maps, list(range(N_CORES)))
    if _results_out is not None:
        _results_out.append(res)
    outs = []
    for c in range(N_CORES):
        oc = res.results[c]["out"].reshape(SS, B, D).transpose(1, 0, 2)
        outs.append(oc)
    return np.ascontiguousarray(np.concatenate(outs, axis=1))
